# revision 1
# baseline (speedup 1.0000x reference)
"""Bass/Trainium2 kernel for a 2-layer GCN (PyG GCNConv x2 with relu between).

Math (reference):
    A~ = A + I (self loops), deg = in-degree of A~, dis = deg^-0.5
    layer(x, W, b) = dis * (A~^T @ (dis * x) @ W) + b   (aggregation over incoming edges)
    out = layer2(relu(layer1(x, W1, b1)), W2, b2)

Factorization used here: the symmetric normalization is folded into per-node
row scalings (dis), so edge aggregation is a pure unweighted gather +
segment-sum, and the dense 64x64 matmul is applied after aggregation
(associativity: A~(xW) = (A~x)W).

Distribution: nodes are dealt to 8 cores in degree-sorted round-robin order
(equalizes per-block degree profiles across cores so the shared SPMD schedule
pads minimally). Edges are partitioned by target core. Each core:
  - gathers source rows of a replicated table (HBM) per edge token via
    dma_gather (4 table chunks of 25088 rows to satisfy the int16 index range)
  - segment-sums 128-token windows on the tensor engine using one-hot masks
    built by the vector engine (is_equal of seg ids vs an iota row)
  - accumulates feature-major partial sums in PSUM groups of 8 target blocks,
    flushing additively to an SBUF accumulator Zt
  - applies the 64x64 weight matmul, bias, relu and dis scaling per block.
Two launches (one per layer); the host concatenates the per-core H' shards
into the layer-2 table between launches.
"""

import os
import numpy as np
import concourse.bass as bass
import concourse.bacc as bacc
import concourse.mybir as mybir
from concourse.tile import TileContext
from concourse.bass_utils import run_bass_kernel_spmd

F32 = mybir.dt.float32
BF16 = mybir.dt.bfloat16
I16 = mybir.dt.int16
# precision mode: "split" (bf16 hi+lo residual rows, near-fp32 accuracy),
# "bf16" (fastest), "fp32" (exact, PE-bound)
MODE = os.environ.get("GCN_MODE", "split")
USE_BF16 = MODE in ("bf16", "split")
USE_SPLIT = MODE == "split"



class Cfg:
    def __init__(self, n_nodes, cores=8, call_t=8192):
        self.N = n_nodes
        self.CORES = cores
        self.D = 64
        self.NPC = n_nodes // cores            # real nodes per core
        self.NBLK = (self.NPC + 127) // 128    # target blocks per core
        self.NPAD = self.NBLK * 128            # padded targets per core
        self.TROWS = self.NPAD * cores         # table rows
        assert self.TROWS % 4 == 0
        self.CHUNKR = self.TROWS // 4          # rows per gather chunk
        assert self.CHUNKR <= 32768
        self.GRPB = 8                          # blocks per psum group
        self.NGRP = (self.NBLK + 7) // 8
        self.CALL_T = call_t                   # max tokens per dma_gather call
        self.MASKW = 16                        # windows per mask tile


FULL = Cfg(100000)


# ---------------------------------------------------------------- host prep
def _prepare(cfg, edge_index):
    """Build per-core token streams and the shared SPMD schedule."""
    src = np.asarray(edge_index[0], dtype=np.int64)
    tgt = np.asarray(edge_index[1], dtype=np.int64)
    N, C = cfg.N, cfg.CORES

    deg = np.bincount(tgt, minlength=N).astype(np.int64) + 1
    dis = (deg.astype(np.float32)) ** np.float32(-0.5)

    # degree-sorted round-robin deal: rank i -> core i%C (equalizes per-core
    # degree profiles so the shared SPMD schedule pads minimally)
    order = np.argsort(deg, kind="stable")
    node_core = np.empty(N, np.int32)
    ranks = np.arange(N)
    node_core[order] = (ranks % C).astype(np.int32)

    # per-core LPT packing of targets into 128-slot blocks with near-equal
    # token sums (deg+1 incl self loop), sized so each (block, chunk) cell
    # lands just under a multiple of 128 tokens
    tokens = deg  # per-target token count (in-edges + self loop)
    per_core_total = max(int(tokens[node_core == q].sum()) for q in range(C))
    nblk = max((cfg.NPC + 127) // 128, -(-per_core_total // 1840))
    cfg.NBLK = nblk
    cfg.NPAD = nblk * 128
    cfg.TROWS = cfg.NPAD * C
    cfg.CHUNKR = cfg.TROWS // 4
    assert cfg.CHUNKR <= 32768, cfg.CHUNKR
    cfg.NGRP = (nblk + cfg.GRPB - 1) // cfg.GRPB

    # serpentine deal of degree-sorted targets over blocks: balances block
    # token sums and cardinality (<= ceil(NPC/nblk)+1 <= 128 targets/block)
    node_loc = np.empty(N, np.int32)
    for q in range(C):
        own = np.flatnonzero(node_core == q)
        o2 = own[np.argsort(-tokens[own], kind="stable")]
        slot_of_block = np.zeros(nblk, np.int32)
        for r in range(0, len(o2), nblk):
            chunk_nodes = o2[r : r + nblk]
            blocks = np.arange(len(chunk_nodes))
            if (r // nblk) % 2 == 1:
                blocks = nblk - 1 - blocks
            node_loc[chunk_nodes] = blocks * 128 + slot_of_block[blocks]
            slot_of_block[blocks] += 1
        assert slot_of_block.max() <= 128
    # table row: interleave locs over the 4 gather chunks so self-loop and
    # edge tokens of every core spread evenly across chunks
    qtr = cfg.NPAD // 4
    node_loc64 = node_loc.astype(np.int64)
    trow = (
        (node_loc64 % 4) * cfg.CHUNKR
        + node_core.astype(np.int64) * qtr
        + node_loc64 // 4
    )

    # per-core edge lists (edges by target core) + self loops
    e_src_row = trow[src]
    e_tcore = node_core[tgt]
    e_tloc = node_loc[tgt]
    sl_src_row = trow  # self loop src row for node n
    sl_tcore = node_core
    sl_tloc = node_loc

    all_srcrow = np.concatenate([e_src_row, sl_src_row])
    all_tcore = np.concatenate([e_tcore, sl_tcore])
    all_tloc = np.concatenate([e_tloc, sl_tloc])
    chunk = (all_srcrow // cfg.CHUNKR).astype(np.int32)
    block = (all_tloc // 128).astype(np.int32)

    # counts per (core, chunk, block)
    counts = np.zeros((C, 4, cfg.NBLK), np.int64)
    np.add.at(counts, (all_tcore, chunk, block), 1)
    n_win = np.maximum(1, (counts.max(axis=0) + 127) // 128)  # [4, NBLK]
    total_real = int(counts.sum())
    total_slots = int(n_win.sum()) * 128 * 1  # per core
    pad_frac = (total_slots * C - total_real) / max(total_real, 1)

    # token stream layout (shared): chunk-major, block-ascending
    # windows meta: list of (chunk, blk)
    windows = []
    for c in range(4):
        for b in range(cfg.NBLK):
            windows += [(c, b)] * int(n_win[c, b])
    W_total = len(windows)
    T_total = W_total * 128

    # gather calls: contiguous token ranges within one chunk, <= CALL_T
    chunk_w_starts = []
    w0 = 0
    for c in range(4):
        nw = int(n_win[c].sum())
        chunk_w_starts.append((w0, w0 + nw))
        w0 += nw
    calls = []  # (chunk, tok_start, ntok)
    for c, (ws, we) in enumerate(chunk_w_starts):
        t0, t1 = ws * 128, we * 128
        t = t0
        while t < t1:
            n = min(cfg.CALL_T, t1 - t)
            calls.append((c, t, n))
            t += n

    # per-core idx/segrel arrays
    # slot offsets per (chunk, block): window start index
    win_start = np.zeros((4, cfg.NBLK), np.int64)
    acc = 0
    for c in range(4):
        for b in range(cfg.NBLK):
            win_start[c, b] = acc
            acc += int(n_win[c, b])

    per_core = []
    for q in range(C):
        m = all_tcore == q
        csrc = all_srcrow[m]
        cchunk = chunk[m]
        ctloc = all_tloc[m]
        cblk = block[m]
        # sort by (chunk, tloc) then stable
        so = np.lexsort((ctloc, cchunk))
        csrc, cchunk, ctloc, cblk = csrc[so], cchunk[so], ctloc[so], cblk[so]

        idx16 = np.zeros(T_total, np.int16)
        segrel = np.full(T_total, -1, np.int16)
        # place tokens of each (chunk, block) run at its window slots
        # compute run boundaries
        keys = cchunk.astype(np.int64) * cfg.NBLK + cblk
        boundaries = np.flatnonzero(np.diff(keys)) + 1
        starts = np.concatenate([[0], boundaries])
        ends = np.concatenate([boundaries, [len(keys)]])
        for s, e in zip(starts, ends):
            c = int(cchunk[s])
            b = int(cblk[s])
            base = int(win_start[c, b]) * 128
            n = e - s
            idx16[base : base + n] = (csrc[s:e] - c * cfg.CHUNKR).astype(np.int16)
            segrel[base : base + n] = (ctloc[s:e] - b * 128).astype(np.int16)
            # pad tokens: idx 0 (valid row of this chunk), segrel stays -1
        # wrapped idx layout [128, T/16]: idx j at [j%16 (+16k replicas), j//16]
        idxw = np.tile(idx16.reshape(T_total // 16, 16).T, (8, 1)).copy()
        # segrel layout [128, T/128]: token w*128+p at [p, w]
        segw = segrel.reshape(W_total, 128).T.copy()
        # dis per block layout [128, NBLK]: target b*128+p at [p, b]
        disq = np.ones(cfg.NPAD, np.float32)
        own = np.flatnonzero(node_core == q)
        disq[node_loc[own]] = dis[own]
        disb = disq.reshape(cfg.NBLK, 128).T.copy()
        per_core.append(dict(idxw=idxw, segw=segw, disb=disb))

    meta = dict(
        windows=windows,
        calls=calls,
        n_win=n_win,
        W_total=W_total,
        T_total=T_total,
        pad_frac=pad_frac,
        node_core=node_core,
        node_loc=node_loc,
        trow=trow,
        dis=dis,
        per_core=per_core,
    )
    return meta


# ------------------------------------------------------------- kernel build
def _build_layer_nc(cfg, meta, relu, repeat=1):
    """One GCN layer as a Tile kernel. relu=True for layer 1 (bias inside
    relu, then dis scale fused via relu(dis*x)); relu=False for layer 2
    (dis scale then bias)."""
    nc = bacc.Bacc(None, target_bir_lowering=False)
    T, Wn = meta["T_total"], meta["W_total"]
    D, NBLK, NGRP = cfg.D, cfg.NBLK, cfg.NGRP

    MDT = BF16 if USE_BF16 else F32
    table = nc.declare_dram_parameter("table", [cfg.TROWS, D], F32, isOutput=False)
    idxw_d = nc.declare_dram_parameter("idxw", [128, T // 16], I16, isOutput=False)
    segw_d = nc.declare_dram_parameter("segw", [128, Wn], I16, isOutput=False)
    disb_d = nc.declare_dram_parameter("disb", [128, NBLK], F32, isOutput=False)
    bt_d = nc.declare_dram_parameter("bt", [128, D], F32, isOutput=False)
    w_d = nc.declare_dram_parameter("w", [D, D], F32, isOutput=False)
    hout = nc.declare_dram_parameter("hout", [cfg.NPAD, D], F32, isOutput=True)

    windows = meta["windows"]
    calls = meta["calls"]

    with TileContext(nc) as tc:
        with (
            tc.tile_pool(name="const", bufs=1) as cpool,
            tc.tile_pool(name="msg", bufs=3) as mpool,
            tc.tile_pool(name="mask", bufs=3) as kpool,
            tc.tile_pool(name="acc", bufs=1) as apool,
            tc.tile_pool(name="pg", bufs=3, space="PSUM") as pgpool,
            tc.tile_pool(name="p2", bufs=2, space="PSUM") as p2pool,
        ):
            # constants / whole-stream loads
            idxw = cpool.tile([128, T // 16], I16)
            nc.sync.dma_start(out=idxw[:], in_=idxw_d[:])
            segw = cpool.tile([128, Wn], I16)
            nc.sync.dma_start(out=segw[:], in_=segw_d[:])
            disb = cpool.tile([128, NBLK], F32)
            nc.sync.dma_start(out=disb[:], in_=disb_d[:])
            bt = cpool.tile([128, D], F32)
            nc.sync.dma_start(out=bt[:], in_=bt_d[:])
            wt = cpool.tile([128, D], F32)
            nc.sync.dma_start(out=wt[0:D, :], in_=w_d[:])
            nc.sync.dma_start(out=wt[D : 2 * D, :], in_=w_d[:])
            iota = cpool.tile([128, 128], I16)
            nc.gpsimd.iota(iota[:], pattern=[[1, 128]], base=0, channel_multiplier=0)

            iota_exp = cpool.tile([128, 128, cfg.MASKW], I16)
            nc.gpsimd.iota(
                iota_exp[:], pattern=[[1, 128], [0, cfg.MASKW]], base=0,
                channel_multiplier=0,
            )

            for _rep in range(repeat):
                # two SBUF accumulators: A holds chunks 0+1, B holds 2+3;
                # chunk 0/2 flushes are ACT copies (init), 1/3 are DVE adds
                zta = cpool.tile([128, NGRP * 512], F32, tag="zta")
                ztb = cpool.tile([128, NGRP * 512], F32, tag="ztb")

                call_i = 0
                msg_tile = None
                msg_base = 0
                mask_tile = None
                mask_base = 0
                cur_grp = None  # (chunk, grp)
                grp_tile = None
                grp_started = set()

                def flush_grp():
                    c, g = cur_grp
                    zt = zta if c < 2 else ztb
                    init = c % 2 == 0
                    nb = min(NBLK - g * cfg.GRPB, cfg.GRPB)
                    rects = [(slice(0, 64), 128 * min(nb, 4))]
                    if nb > 4:
                        rects.append((slice(64, 128), 128 * (nb - 4)))
                    for rows, wid in rects:
                        dst = zt[rows, g * 512 : g * 512 + wid]
                        if init:
                            nc.scalar.activation(
                                out=dst,
                                in_=grp_tile[rows, 0:wid],
                                func=mybir.ActivationFunctionType.Copy,
                            )
                        else:
                            nc.vector.tensor_tensor(
                                out=dst, in0=dst, in1=grp_tile[rows, 0:wid],
                                op=mybir.AluOpType.add,
                            )

                for w, (c, b) in enumerate(windows):
                    tok = w * 128
                    # new gather call?
                    if call_i < len(calls) and calls[call_i][1] == tok:
                        cc, t0, ntok = calls[call_i]
                        nslots = ntok // 128
                        msg_tile = mpool.tile(
                            [128, cfg.CALL_T // 128, D], F32, tag="msg"
                        )
                        if os.environ.get("SKIP_GATHER"):
                            nc.any.memset(msg_tile[:, :nslots, :], 0.0)
                        else:
                            nc.gpsimd.dma_gather(
                                msg_tile[:, :nslots, :],
                                table[cc * cfg.CHUNKR : (cc + 1) * cfg.CHUNKR, :],
                                idxw[:, t0 // 16 : (t0 + ntok) // 16],
                                num_idxs=ntok,
                                num_idxs_reg=ntok,
                                elem_size=D,
                                single_packet=False,
                            )
                        msg_base = t0
                        call_i += 1
                    # new mask group? (window-minor layout [p, col, w] keeps
                    # every operand innermost-unit-stride for the DVE 2x mode)
                    if mask_tile is None or w - mask_base >= cfg.MASKW:
                        nw = min(cfg.MASKW, Wn - w)
                        mask_tile = kpool.tile([128, 128, cfg.MASKW], MDT, tag="mask")
                        nc.vector.tensor_tensor(
                            out=mask_tile[:, :, :nw],
                            in0=segw[:, w : w + nw]
                            .rearrange("p (o w) -> p o w", o=1)
                            .to_broadcast([128, 128, nw]),
                            in1=iota_exp[:, :, :nw],
                            op=mybir.AluOpType.is_equal,
                        )
                        mask_base = w
                    # new psum group?
                    g = b // cfg.GRPB
                    if cur_grp != (c, g):
                        if cur_grp is not None:
                            flush_grp()
                        grp_tile = pgpool.tile([128, 512], F32, tag="pg")
                        cur_grp = (c, g)
                        grp_started = set()
                    # window matmul; one PSUM accumulation group per (c, b)
                    bg = b % cfg.GRPB
                    h = bg // 4
                    m = bg % 4
                    first = b not in grp_started
                    grp_started.add(b)
                    last = (w + 1 >= len(windows)) or windows[w + 1] != (c, b)
                    out_ap = grp_tile[64 * h : 64 * h + 64, 128 * m : 128 * m + 128]
                    rhs_ap = mask_tile[:, :, w - mask_base]
                    if USE_BF16:
                        mview = msg_tile[:, (tok - msg_base) // 128, :].bitcast(BF16)
                        nc.tensor.matmul(
                            out=out_ap, lhsT=mview[:, 0:D], rhs=rhs_ap,
                            start=first, stop=last and not USE_SPLIT,
                        )
                        if USE_SPLIT:
                            nc.tensor.matmul(
                                out=out_ap, lhsT=mview[:, D : 2 * D], rhs=rhs_ap,
                                start=False, stop=last,
                            )
                    else:
                        nc.tensor.matmul(
                            out=out_ap,
                            lhsT=msg_tile[:, (tok - msg_base) // 128, :],
                            rhs=rhs_ap,
                            start=first,
                            stop=last,
                        )
                flush_grp()

                # epilogue: per block (Zta+Ztb) @ W + bias + (relu) + dis scale
                stage = apool.tile([128, NBLK, D], F32)
                for b in range(NBLK):
                    bg = b % cfg.GRPB
                    g, h, m = b // cfg.GRPB, bg // 4, bg % 4
                    cols = slice(g * 512 + 128 * m, g * 512 + 128 * m + 128)
                    ps2 = p2pool.tile([128, D], F32, tag="p2")
                    nc.tensor.matmul(
                        out=ps2[:],
                        lhsT=zta[64 * h : 64 * h + 64, cols],
                        rhs=wt[64 * h : 64 * h + 64, :],
                        start=True,
                        stop=False,
                    )
                    nc.tensor.matmul(
                        out=ps2[:],
                        lhsT=ztb[64 * h : 64 * h + 64, cols],
                        rhs=wt[64 * h : 64 * h + 64, :],
                        start=False,
                        stop=True,
                    )
                    if relu:
                        # H' = dis * relu(dis*(Z@W1) + b1)
                        sc = mpool.tile([128, D], F32, tag="sc")
                        nc.scalar.activation(
                            out=sc[:],
                            in_=ps2[:],
                            func=mybir.ActivationFunctionType.Copy,
                            scale=disb[:, b : b + 1],
                        )
                        tmp = mpool.tile([128, D], F32, tag="tmp")
                        nc.vector.tensor_tensor(
                            out=tmp[:], in0=sc[:], in1=bt[:], op=mybir.AluOpType.add
                        )
                        # relu(dis * t) == dis * relu(t) since dis > 0
                        nc.scalar.activation(
                            out=stage[:, b, :],
                            in_=tmp[:],
                            func=mybir.ActivationFunctionType.Relu,
                            scale=disb[:, b : b + 1],
                        )
                    else:
                        tmp = mpool.tile([128, D], F32, tag="tmp")
                        nc.scalar.activation(
                            out=tmp[:],
                            in_=ps2[:],
                            func=mybir.ActivationFunctionType.Copy,
                            scale=disb[:, b : b + 1],
                        )
                        nc.vector.tensor_tensor(
                            out=stage[:, b, :], in0=tmp[:], in1=bt[:],
                            op=mybir.AluOpType.add,
                        )
                nc.sync.dma_start(
                    out=hout[:].rearrange("(b p) d -> p b d", p=128), in_=stage[:]
                )

    nc.compile()
    return nc


# ---------------------------------------------------------------- execution
_CACHE = {}


def _get_built(cfg, meta):
    key = ("nc", cfg.N, meta["W_total"])
    if key not in _CACHE:
        _CACHE[key] = (
            _build_layer_nc(cfg, meta, relu=True),
            _build_layer_nc(cfg, meta, relu=False),
        )
    return _CACHE[key]


def _run_layer(nc, cfg, meta, table, wmat, bvec, trace=False):
    if table.dtype != np.float32:
        table = table.view(np.float32)
    bt = np.tile(bvec[None, :], (128, 1)).astype(np.float32)
    in_maps = []
    for q in range(cfg.CORES):
        pc = meta["per_core"][q]
        in_maps.append(
            dict(
                table=table,
                idxw=pc["idxw"],
                segw=pc["segw"],
                disb=pc["disb"],
                bt=bt,
                w=np.ascontiguousarray(wmat, dtype=np.float32),
            )
        )
    res = run_bass_kernel_spmd(
        nc, in_maps, core_ids=list(range(cfg.CORES)), trace=trace
    )
    shards = [res.results[q]["hout"] for q in range(cfg.CORES)]
    return shards, res


def gcn_forward(cfg, x, edge_index, W1, b1, W2, b2, trace=False):
    key = ("meta", cfg.N, int(np.asarray(edge_index).sum()) & 0xFFFFFFFF)
    if key not in _CACHE:
        _CACHE[key] = _prepare(cfg, edge_index)
    meta = _CACHE[key]
    nc1, nc2 = _get_built(cfg, meta)

    dis = meta["dis"]
    trow = meta["trow"]
    xp = np.asarray(x, np.float32) * dis[:, None]
    if USE_BF16:
        import ml_dtypes
        table1 = np.zeros((cfg.TROWS, 128), ml_dtypes.bfloat16)
        hi = xp.astype(ml_dtypes.bfloat16)
        table1[trow, : cfg.D] = hi
        if USE_SPLIT:
            table1[trow, cfg.D :] = (xp - hi.astype(np.float32)).astype(
                ml_dtypes.bfloat16
            )
    else:
        table1 = np.zeros((cfg.TROWS, cfg.D), np.float32)
        table1[trow] = xp

    shards1, res1 = _run_layer(nc1, cfg, meta, table1, W1, b1, trace=trace)
    locs = np.arange(cfg.NPAD, dtype=np.int64)
    qtr = cfg.NPAD // 4
    if USE_BF16:
        import ml_dtypes
        table2 = np.zeros((cfg.TROWS, 128), ml_dtypes.bfloat16)
        for q in range(cfg.CORES):
            rows = (locs % 4) * cfg.CHUNKR + q * qtr + locs // 4
            hi = shards1[q].astype(ml_dtypes.bfloat16)
            table2[rows, : cfg.D] = hi
            if USE_SPLIT:
                table2[rows, cfg.D :] = (
                    shards1[q] - hi.astype(np.float32)
                ).astype(ml_dtypes.bfloat16)
    else:
        table2 = np.zeros((cfg.TROWS, cfg.D), np.float32)
        for q in range(cfg.CORES):
            rows = (locs % 4) * cfg.CHUNKR + q * qtr + locs // 4
            table2[rows] = shards1[q]
    shards2, res2 = _run_layer(nc2, cfg, meta, table2, W2, b2, trace=trace)

    out = np.empty((cfg.N, cfg.D), np.float32)
    nc_, nl_ = meta["node_core"], meta["node_loc"]
    allsh = np.concatenate(shards2, axis=0)
    out[:] = allsh[nc_.astype(np.int64) * cfg.NPAD + nl_]
    return out, (res1, res2)


def kernel(x, edge_index, W1, b1, W2, b2):
    out, _ = gcn_forward(
        FULL,
        np.asarray(x),
        np.asarray(edge_index),
        np.asarray(W1),
        np.asarray(b1),
        np.asarray(W2),
        np.asarray(b2),
    )
    return out

